# revision 12
# baseline (speedup 1.0000x reference)
"""Trainium2 Bass kernel for a SqueezeNet Fire module.

    x [32, 512, 56, 56] fp32
    s  = relu(squeeze_w @ x + squeeze_b)          # 1x1, 512 -> 64
    e1 = relu(expand1x1_w @ s + expand1x1_b)      # 1x1, 64 -> 256
    e3 = relu(conv3x3(s, expand3x3_w) + b)        # 3x3 pad 1, 64 -> 256
    out = concat([e1, e3], channel)               # [32, 512, 56, 56] fp32

Sharding: data-parallel over batch, 4 images per NeuronCore x 8 cores.

Per-core plan (per image, spatial flattened to 56x56=3136, chunked 7x448):
  - squeeze: 4 accumulating K=128 matmuls. The stationary weights are
    duplicated along M (64 real channels -> 128) so PSUM rows 0-63 and 64-127
    both hold S; one relu+bias eviction writes both halves of a zero-padded
    S buffer SS [128, 58, 58] (partitions 0-63 = copy A, 64-127 = copy B).
  - expand1x1 / expand3x3: K=64 matmuls issued as pairs on row groups 0-63 and
    64-127 (auto tile_position from base_partition) so each pair runs
    concurrently in the PE array. expand3x3 = 9 shifted-tap matmuls
    accumulating in PSUM, taps read shifted windows of SS.

Trace-driven schedule (based on NTFF profiles of this kernel):
  - The PE is the bottleneck in steady state; switching the stationary
    K-config (128-row squeeze <-> 64-row expand pairs) costs ~98ns of
    pipeline drain, so squeeze is emitted in 2-chunk batches (2 switches
    per 2 chunks instead of 4). Squeeze runs 4-5 chunks ahead of expand.
  - The PE p-state ramps from 0.65 to 2.4 GHz over ~3us of activity;
    warm-up matmuls on a zeroed dummy tile run during the DMA fill so real
    work starts at full clock.
  - The fill is DMA-latency-bound: x chunk loads for image 0 are split in
    half and spread over all three DMA issue paths (sync + scalar HWDGE,
    gpsimd SWDGE) so the squeeze never starves; weights load concurrently
    (wsq first on sync; w1/w3/bias on scalar).
  - Tail: the last chunk's e3 eviction is split ACT/DVE and the final
    output group's 4 DMAs go to 3 different queues.

I/O is staged in bf16 (x cast on host, output upcast on host); matmul
operands bf16 with fp32 PSUM accumulation.
"""

import sys

if "/opt/trn_rl_repo" not in sys.path:
    sys.path.insert(0, "/opt/trn_rl_repo")

import ml_dtypes
import numpy as np

import concourse.bass as bass
import concourse.tile as tile
from concourse import bacc, mybir

F32 = mybir.dt.float32
F32R = mybir.dt.float32r
BF16 = mybir.dt.bfloat16
RELU = mybir.ActivationFunctionType.Relu

N_CORES = 8
N_TOTAL, C_IN, H, W = 32, 512, 56, 56
N_IMG = N_TOTAL // N_CORES          # images per core
C_SQ, C_E = 64, 256                 # squeeze / expand channels
HW = H * W                          # 3136
ROWS_PER_CHUNK = 8
N_CHUNK = H // ROWS_PER_CHUNK       # 7 chunks of 8 rows
CHUNK = ROWS_PER_CHUNK * W          # 448 spatial positions per chunk
HP, WP = H + 2, W + 2               # padded S frame 58x58
K_TILES = C_IN // 128               # 4

IN_BF16 = True                      # ship x to the device as bf16
EXP_BF16 = True                     # expand path (S buffer + weights) in bf16
OUT_BF16 = True                     # write out as bf16, upcast on host

N_WARM = 8                          # PE p-state warm-up matmuls (448 cols)


def _build(in_bf16, exp_bf16, out_bf16):
    xdt = BF16 if in_bf16 else F32R
    edt = BF16 if exp_bf16 else F32R
    odt = BF16 if out_bf16 else F32
    nc = bacc.Bacc("TRN2", target_bir_lowering=False, debug=False,
                   num_devices=N_CORES)
    x_d = nc.dram_tensor("x", [N_IMG, 128, K_TILES, HW], xdt,
                         kind="ExternalInput").ap()
    wsq_d = nc.dram_tensor("wsq", [128, K_TILES, 128], xdt,
                           kind="ExternalInput").ap()
    w1_d = nc.dram_tensor("w1", [128, 128], edt, kind="ExternalInput").ap()
    w3_d = nc.dram_tensor("w3", [128, 9, 128], edt, kind="ExternalInput").ap()
    bias_d = nc.dram_tensor("bias", [128, 5], F32, kind="ExternalInput").ap()
    out_d = nc.dram_tensor("out", [N_IMG, 2 * C_E, HW], odt,
                           kind="ExternalOutput").ap()

    with tile.TileContext(nc) as tc:
        with (
            tc.tile_pool(name="wpool", bufs=1) as wpool,
            tc.tile_pool(name="xpool", bufs=16) as xpool,
            tc.tile_pool(name="sspool", bufs=2) as sspool,
            tc.tile_pool(name="opool", bufs=4) as opool,
            tc.tile_pool(name="psum", bufs=1, space="PSUM") as psum,
        ):
            wsq_t = wpool.tile([128, K_TILES, 128], xdt)
            w1_t = wpool.tile([128, 128], edt)
            w3_t = wpool.tile([128, 9, 128], edt)
            bias_t = wpool.tile([128, 5], F32)
            bsq_t = bias_t[:, 0:1]
            b1_t = bias_t[:, 1:3]
            b3_t = bias_t[:, 3:5]

            # PE warm-up operands: a zeroed stationary + moving tile. The
            # warm-up matmuls carry no data deps, so they run during the
            # DMA fill and pull the PE out of its low p-state (~3us ramp).
            wdum = wpool.tile([128, 128], xdt)
            xdum = wpool.tile([128, CHUNK], xdt)
            mdt = BF16 if in_bf16 else F32
            nc.vector.memset(wdum.bitcast(mdt), 0.0)
            nc.vector.memset(xdum.bitcast(mdt), 0.0)

            # warm the scalar engine's activation table early — otherwise
            # the ~1.3us ACT_TABLE_LOAD fires lazily on the first squeeze
            # eviction, in the pipeline's critical path
            warm = wpool.tile([1, 1], F32)
            nc.vector.memset(warm[:], 0.0)
            nc.scalar.activation(warm[:], warm[:], RELU)

            x_tiles = {}    # (image, chunk group) -> [128, K_TILES, 2*CHUNK]
            ss_tiles = {}   # image -> SS tile
            out_stage = [None] * 4

            def group_tile(n, g):
                w = min(2 * CHUNK, HW - 2 * g * CHUNK)
                t = xpool.tile([128, K_TILES, w], xdt, tag="xc",
                               name=f"xc_{n}_{g}")
                x_tiles[(n, g)] = t
                return t

            def load_piece(eng, n, j, khalf):
                # one [128, 2, 448] quadrant of a chunk on the given queue
                g, half = j // 2, j % 2
                t = x_tiles.get((n, g))
                if t is None:
                    t = group_tile(n, g)
                c0 = half * CHUNK
                k0 = khalf * 2
                eng.dma_start(
                    t[:, k0 : k0 + 2, c0 : c0 + CHUNK],
                    x_d[n, :, k0 : k0 + 2,
                        j * CHUNK : (j + 1) * CHUNK],
                )

            def load_chunk(eng, n, j):
                g, half = j // 2, j % 2
                t = x_tiles.get((n, g))
                if t is None:
                    t = group_tile(n, g)
                c0 = half * CHUNK
                eng.dma_start(
                    t[:, :, c0 : c0 + CHUNK],
                    x_d[n, :, :, j * CHUNK : (j + 1) * CHUNK],
                )

            def load_group(n, g, eng=None):
                eng = eng or nc.gpsimd
                w = min(2 * CHUNK, HW - 2 * g * CHUNK)
                t = group_tile(n, g)
                eng.dma_start(t[:], x_d[n, :, :, 2 * g * CHUNK : 2 * g * CHUNK + w])

            # ---- fill ----
            # The scalar (ACT) HWDGE queue measures ~70 B/ns — 3x slower
            # than the sync HWDGE / gpsimd SWDGE queues (~200 B/ns each) —
            # so it only carries the small weight tensors. x is strictly
            # interleaved by consumption deadline across sync + gpsimd
            # (k01 half on sync, k23 half on gpsimd for chunks 0-3).
            nc.sync.dma_start(wsq_t[:], wsq_d[:])
            nc.scalar.dma_start(bias_t[:], bias_d[:])
            load_piece(nc.gpsimd, 0, 0, 1)     # c0 k23
            nc.scalar.dma_start(w1_t[:], w1_d[:])
            load_piece(nc.sync, 0, 0, 0)       # c0 k01
            load_piece(nc.gpsimd, 0, 1, 1)     # c1 k23
            load_piece(nc.sync, 0, 1, 0)       # c1 k01
            nc.scalar.dma_start(w3_t[:], w3_d[:])
            load_piece(nc.gpsimd, 0, 2, 1)     # c2 k23
            load_piece(nc.sync, 0, 2, 0)       # c2 k01
            load_piece(nc.gpsimd, 0, 3, 1)     # c3 k23
            load_piece(nc.sync, 0, 3, 0)       # c3 k01
            load_chunk(nc.gpsimd, 0, 5)
            load_chunk(nc.sync, 0, 4)
            load_chunk(nc.gpsimd, 0, 6)
            load_chunk(nc.sync, 1, 1)
            load_chunk(nc.gpsimd, 1, 0)
            load_group(1, 1, eng=nc.sync)      # chunks (1,2),(1,3)
            # Remaining groups issue in-loop, paced ~5 groups ahead of
            # consumption. Two constraints meet here: every group has its
            # own SBUF buffer (xpool bufs=16) so loads never wait on PE
            # progress (WAR), but the tile framework recycles DMA
            # semaphores with cumulative thresholds, so issuing far ahead
            # of consumption creates false waits on much-later DMAs.
            rest_groups = [(1, 2), (1, 3), (2, 0), (2, 1), (2, 2), (2, 3),
                           (3, 0), (3, 1), (3, 2), (3, 3)]
            first_use = {g: 7 * g[0] + 2 * g[1] for g in rest_groups}

            # ---- PE warm-up ----
            for d in range(N_WARM):
                ps = psum.tile([128, ROWS_PER_CHUNK, W], F32, tag="sq", bufs=2,
                               name=f"warm{d}")
                nc.tensor.matmul(ps[:], wdum[:], xdum[:], start=True, stop=True)

            def setup_image(n):
                ss = sspool.tile([128, HP, WP], edt, tag="ss")
                # zero the one-pixel border of the padded S frame (memset
                # rejects the f32r dtype tag, so write through a plain view)
                sdt = BF16 if exp_bf16 else F32
                nc.vector.memset(ss[:, 0, :].bitcast(sdt), 0.0)
                nc.vector.memset(ss[:, HP - 1, :].bitcast(sdt), 0.0)
                nc.vector.memset(ss[:, 1 : HP - 1, 0].bitcast(sdt), 0.0)
                nc.vector.memset(ss[:, 1 : HP - 1, WP - 1].bitcast(sdt), 0.0)
                ss_tiles[n] = ss

            def squeeze_chunk(n, j):
                if n not in ss_tiles:
                    setup_image(n)
                ps = psum.tile([128, ROWS_PER_CHUNK, W], F32, tag="sq", bufs=2,
                               name=f"sq_{n}_{j}")
                xt = x_tiles[(n, j // 2)]
                c0 = (j % 2) * CHUNK
                for k in range(K_TILES):
                    nc.tensor.matmul(
                        ps[:],
                        wsq_t[:, k, :],
                        xt[:, k, c0 : c0 + CHUNK],
                        start=(k == 0),
                        stop=(k == K_TILES - 1),
                    )
                # relu+bias eviction into both duplicated halves of SS
                # interior; alternate DVE/ACT by chunk parity so consecutive
                # evictions overlap instead of queuing on one engine.
                # (even chunks -> DVE: the scalar engine is still issuing
                # fill DMAs when the first eviction retires)
                y0 = j * ROWS_PER_CHUNK
                dst = ss_tiles[n][:, 1 + y0 : 1 + y0 + ROWS_PER_CHUNK, 1 : 1 + W]
                if j % 2 == 0:
                    nc.vector.tensor_scalar(
                        dst, ps[:], bsq_t, 0.0,
                        op0=mybir.AluOpType.add, op1=mybir.AluOpType.max,
                    )
                else:
                    nc.scalar.activation(dst, ps[:], RELU, bias=bsq_t)

            e_state = {}

            def expand_chunk_mm(n, j):
                ss = ss_tiles[n]
                y0 = j * ROWS_PER_CHUNK
                p1 = [psum.tile([128, CHUNK], F32, tag=f"e1h{h}", bufs=1,
                                name=f"p1h{h}_{n}_{j}")
                      for h in range(2)]
                p3 = [psum.tile([128, CHUNK], F32, tag=f"e3h{h}", bufs=2,
                                name=f"p3h{h}_{n}_{j}")
                      for h in range(2)]
                e_state[(n, j)] = (p1, p3)
                # expand1x1: one K=64 matmul per half, concurrent pair
                for h in range(2):
                    nc.tensor.matmul(
                        p1[h][:],
                        w1_t[64 * h : 64 * h + 64, :],
                        ss[64 * h : 64 * h + 64,
                           1 + y0 : 1 + y0 + ROWS_PER_CHUNK, 1 : 1 + W],
                        start=True,
                        stop=True,
                    )
                # expand3x3: 9 shifted taps accumulate; h0/h1 issued as pairs
                for t in range(9):
                    dy, dx = t // 3, t % 3
                    for h in range(2):
                        nc.tensor.matmul(
                            p3[h][:],
                            w3_t[64 * h : 64 * h + 64, t, :],
                            ss[64 * h : 64 * h + 64,
                               y0 + dy : y0 + dy + ROWS_PER_CHUNK,
                               dx : dx + W],
                            start=(t == 0),
                            stop=(t == 8),
                        )

            def expand_chunk_evict(n, j, last=False):
                p1, p3 = e_state.pop((n, j))
                # evictions: e1 on vector engine, e3 on scalar engine (the
                # last chunk's second e3 half goes to DVE to shorten the
                # drain). Outputs stage in 2-chunk tiles; one DMA per role
                # per pair of chunks (issued after the odd chunk's eviction).
                g, half = j // 2, j % 2
                gw = 1 if j == N_CHUNK - 1 else 2   # odd last chunk: solo group
                if half == 0:
                    for role in range(4):
                        out_stage[role] = opool.tile(
                            [128, gw, CHUNK], odt, tag=f"o{role}",
                            name=f"o{role}_{n}_{g}")
                for h in range(2):
                    nc.vector.tensor_scalar(
                        out_stage[h][:, half, :], p1[h][:],
                        b1_t[:, h : h + 1], 0.0,
                        op0=mybir.AluOpType.add, op1=mybir.AluOpType.max,
                    )
                nc.scalar.activation(out_stage[2][:, half, :],
                                     p3[0][:], RELU, bias=b3_t[:, 0:1])
                if last:
                    nc.vector.tensor_scalar(
                        out_stage[3][:, half, :], p3[1][:],
                        b3_t[:, 1:2], 0.0,
                        op0=mybir.AluOpType.add, op1=mybir.AluOpType.max,
                    )
                else:
                    nc.scalar.activation(out_stage[3][:, half, :],
                                         p3[1][:], RELU, bias=b3_t[:, 1:2])
                if half + 1 == gw:
                    engines = [nc.sync] * 4
                    if last:
                        engines = [nc.sync, nc.sync, nc.sync, nc.gpsimd]
                    for role in range(4):
                        ch0 = 128 * role
                        engines[role].dma_start(
                            out_d[n, ch0 : ch0 + 128,
                                  2 * g * CHUNK : (2 * g + gw) * CHUNK],
                            out_stage[role][:],
                        )

            # Pipeline: squeeze runs 4-5 chunks ahead of expand, emitted in
            # 2-chunk batches so the PE only pays the 64<->128-row stationary
            # reconfiguration twice per 2 chunks. expand(i)'s dy>=1 taps read
            # S rows written by squeeze(i+1)'s eviction; the 4-chunk lead
            # keeps the PE busy while those evictions retire.
            chunks = [(n, j) for n in range(N_IMG) for j in range(N_CHUNK)]
            NC_ALL = len(chunks)
            # Fill ramp at squeeze-lead 2: interleave the first two expands
            # between squeezes so the PE always has expand work (which needs
            # no fresh x) queued behind a possibly-late x chunk. Batches of
            # [sq sq | e e] only from chunk 2 on, once the fill has caught up.
            squeeze_chunk(*chunks[0])
            squeeze_chunk(*chunks[1])
            squeeze_chunk(*chunks[2])
            expand_chunk_mm(*chunks[0])
            expand_chunk_evict(*chunks[0])
            squeeze_chunk(*chunks[3])
            expand_chunk_mm(*chunks[1])
            expand_chunk_evict(*chunks[1])
            next_rest = 0
            for i in range(2, NC_ALL, 2):
                while (next_rest < len(rest_groups)
                       and first_use[rest_groups[next_rest]] < i + 12):
                    load_group(*rest_groups[next_rest])
                    next_rest += 1
                if i + 2 < NC_ALL:
                    squeeze_chunk(*chunks[i + 2])
                if i + 3 < NC_ALL:
                    squeeze_chunk(*chunks[i + 3])
                last = i + 2 >= NC_ALL
                expand_chunk_mm(*chunks[i])
                expand_chunk_evict(*chunks[i], last=False)
                expand_chunk_mm(*chunks[i + 1])
                expand_chunk_evict(*chunks[i + 1], last=last)

    nc.compile()
    return nc


_NC_CACHE = {}


def _get_nc(in_bf16=IN_BF16, exp_bf16=EXP_BF16, out_bf16=OUT_BF16):
    key = (in_bf16, exp_bf16, out_bf16)
    if key not in _NC_CACHE:
        _NC_CACHE[key] = _build(in_bf16, exp_bf16, out_bf16)
    return _NC_CACHE[key]


def _pack_inputs(x, squeeze_w, squeeze_b, expand1x1_w, expand1x1_b,
                 expand3x3_w, expand3x3_b, in_bf16=IN_BF16, exp_bf16=EXP_BF16):
    """Host-side packing of weights into the SBUF-ready layouts."""
    f = np.float32
    xdt = ml_dtypes.bfloat16 if in_bf16 else f
    edt = ml_dtypes.bfloat16 if exp_bf16 else f
    # wsq[p, k, m] = squeeze_w[m % 64, 128k + p]  (M duplicated 64 -> 128)
    wsq = np.ascontiguousarray(
        np.tile(squeeze_w, (2, 1))                 # [128, 512]
        .T.reshape(K_TILES, 128, 128)              # [k, p, m]
        .transpose(1, 0, 2)
    ).astype(xdt)
    # w1[64h + s, m] = expand1x1_w[128h + m, s]
    w1 = np.concatenate(
        [expand1x1_w[:128].T, expand1x1_w[128:].T], axis=0
    ).astype(edt)                                   # [128, 128]
    # w3[64h + s, 3dy + dx, m] = expand3x3_w[128h + m, s, dy, dx]
    w3e = expand3x3_w.reshape(2, 128, C_SQ, 9)      # [h, m, s, t]
    w3 = np.ascontiguousarray(w3e.transpose(0, 2, 3, 1)).reshape(128, 9, 128)
    w3 = w3.astype(edt)
    # bias[p] = [bsq | b1 pair | b3 pair]  (one DMA for all biases)
    bias = np.empty((128, 5), dtype=f)
    bias[:, 0] = np.tile(squeeze_b, 2)
    bias[:, 1:3] = expand1x1_b.reshape(2, 128).T
    bias[:, 3:5] = expand3x3_b.reshape(2, 128).T
    # [cores, n, 128k+p, s] -> [cores, n, p, k, s] so a chunk-group load is
    # one DMA with partition-major layout
    xs = np.ascontiguousarray(
        x.reshape(N_CORES, N_IMG, K_TILES, 128, HW).transpose(0, 1, 3, 2, 4)
    ).astype(xdt)
    return xs, {"wsq": wsq, "w1": w1, "w3": w3, "bias": bias}


def _run(inputs, trace=False, in_bf16=IN_BF16, exp_bf16=EXP_BF16,
         out_bf16=OUT_BF16):
    from concourse import bass_utils

    nc = _get_nc(in_bf16, exp_bf16, out_bf16)
    xs, weights = _pack_inputs(**inputs, in_bf16=in_bf16, exp_bf16=exp_bf16)
    in_maps = [{"x": xs[c], **weights} for c in range(N_CORES)]
    res = bass_utils.run_bass_kernel_spmd(
        nc, in_maps, core_ids=list(range(N_CORES)), trace=trace
    )
    out = np.concatenate([res.results[c]["out"] for c in range(N_CORES)], axis=0)
    return out.reshape(N_TOTAL, 2 * C_E, H, W).astype(np.float32), res


def kernel(**inputs) -> np.ndarray:
    inputs = {k: np.asarray(v, dtype=np.float32) for k, v in inputs.items()}
    out, _ = _run(inputs, trace=False)
    return out


# revision 15
# speedup vs baseline: 1.0598x; 1.0598x over previous
"""Trainium2 Bass kernel for a SqueezeNet Fire module.

    x [32, 512, 56, 56] fp32
    s  = relu(squeeze_w @ x + squeeze_b)          # 1x1, 512 -> 64
    e1 = relu(expand1x1_w @ s + expand1x1_b)      # 1x1, 64 -> 256
    e3 = relu(conv3x3(s, expand3x3_w) + b)        # 3x3 pad 1, 64 -> 256
    out = concat([e1, e3], channel)               # [32, 512, 56, 56] fp32

Sharding: data-parallel over batch, 4 images per NeuronCore x 8 cores.

Per-core plan (per image, spatial flattened to 56x56=3136, chunked 7x448):
  - squeeze: 4 accumulating K=128 matmuls. The stationary weights are
    duplicated along M (64 real channels -> 128) so PSUM rows 0-63 and 64-127
    both hold S; one relu+bias eviction writes both halves of a zero-padded
    S buffer SS [128, 58, 58] (partitions 0-63 = copy A, 64-127 = copy B).
  - expand1x1 / expand3x3: K=64 matmuls issued as pairs on row groups 0-63 and
    64-127 (auto tile_position from base_partition) so each pair runs
    concurrently in the PE array. expand3x3 = 9 shifted-tap matmuls
    accumulating in PSUM, taps read shifted windows of SS.

Trace-driven schedule (based on NTFF profiles of this kernel):
  - The PE is the bottleneck in steady state; switching the stationary
    K-config (128-row squeeze <-> 64-row expand pairs) costs ~98ns of
    pipeline drain, so squeeze is emitted in 2-chunk batches (2 switches
    per 2 chunks instead of 4). Squeeze runs 4-5 chunks ahead of expand.
  - The PE p-state ramps from 0.65 to 2.4 GHz over ~3us of activity;
    warm-up matmuls on a zeroed dummy tile run during the DMA fill so real
    work starts at full clock.
  - The fill is DMA-latency-bound: x chunk loads for image 0 are split in
    half and spread over all three DMA issue paths (sync + scalar HWDGE,
    gpsimd SWDGE) so the squeeze never starves; weights load concurrently
    (wsq first on sync; w1/w3/bias on scalar).
  - Tail: the last chunk's e3 eviction is split ACT/DVE and the final
    output group's 4 DMAs go to 3 different queues.

I/O is staged in bf16 (x cast on host, output upcast on host); matmul
operands bf16 with fp32 PSUM accumulation.
"""

import sys

if "/opt/trn_rl_repo" not in sys.path:
    sys.path.insert(0, "/opt/trn_rl_repo")

import ml_dtypes
import numpy as np

import concourse.bass as bass
import concourse.tile as tile
from concourse import bacc, mybir

F32 = mybir.dt.float32
F32R = mybir.dt.float32r
BF16 = mybir.dt.bfloat16
RELU = mybir.ActivationFunctionType.Relu

N_CORES = 8
N_TOTAL, C_IN, H, W = 32, 512, 56, 56
N_IMG = N_TOTAL // N_CORES          # images per core
C_SQ, C_E = 64, 256                 # squeeze / expand channels
HW = H * W                          # 3136
ROWS_PER_CHUNK = 8
N_CHUNK = H // ROWS_PER_CHUNK       # 7 chunks of 8 rows
CHUNK = ROWS_PER_CHUNK * W          # 448 spatial positions per chunk
HP, WP = H + 2, W + 2               # padded S frame 58x58
K_TILES = C_IN // 128               # 4

IN_BF16 = True                      # ship x to the device as bf16
EXP_BF16 = True                     # expand path (S buffer + weights) in bf16
OUT_BF16 = True                     # write out as bf16, upcast on host

N_WARM = 6                          # PE p-state warm-up matmuls (448 cols)


def _build(in_bf16, exp_bf16, out_bf16):
    xdt = BF16 if in_bf16 else F32R
    edt = BF16 if exp_bf16 else F32R
    odt = BF16 if out_bf16 else F32
    nc = bacc.Bacc("TRN2", target_bir_lowering=False, debug=False,
                   num_devices=N_CORES)
    x_d = nc.dram_tensor("x", [N_IMG, 128, K_TILES, HW], xdt,
                         kind="ExternalInput").ap()
    wsq_d = nc.dram_tensor("wsq", [128, K_TILES, 128], xdt,
                           kind="ExternalInput").ap()
    w1_d = nc.dram_tensor("w1", [128, 128], edt, kind="ExternalInput").ap()
    w3_d = nc.dram_tensor("w3", [128, 9, 128], edt, kind="ExternalInput").ap()
    bias_d = nc.dram_tensor("bias", [128, 5], F32, kind="ExternalInput").ap()
    out_d = nc.dram_tensor("out", [N_IMG, 2 * C_E, HW], odt,
                           kind="ExternalOutput").ap()

    with tile.TileContext(nc) as tc:
        with (
            tc.tile_pool(name="wpool", bufs=1) as wpool,
            tc.tile_pool(name="xpool", bufs=16) as xpool,
            tc.tile_pool(name="sspool", bufs=2) as sspool,
            tc.tile_pool(name="opool", bufs=4) as opool,
            tc.tile_pool(name="psum", bufs=1, space="PSUM") as psum,
        ):
            wsq_t = wpool.tile([128, K_TILES, 128], xdt)
            w1_t = wpool.tile([128, 128], edt)
            w3_t = wpool.tile([128, 9, 128], edt)
            bias_t = wpool.tile([128, 5], F32)
            bsq_t = bias_t[:, 0:1]
            b1_t = bias_t[:, 1:3]
            b3_t = bias_t[:, 3:5]

            # PE warm-up operands: a zeroed stationary + moving tile. The
            # warm-up matmuls carry no data deps, so they run during the
            # DMA fill and pull the PE out of its low p-state (~3us ramp).
            wdum = wpool.tile([128, 128], xdt)
            xdum = wpool.tile([128, CHUNK], xdt)
            mdt = BF16 if in_bf16 else F32
            nc.vector.memset(wdum.bitcast(mdt), 0.0)
            nc.vector.memset(xdum.bitcast(mdt), 0.0)

            # warm the scalar engine's activation table early — otherwise
            # the ~1.3us ACT_TABLE_LOAD fires lazily on the first squeeze
            # eviction, in the pipeline's critical path
            warm = wpool.tile([1, 1], F32)
            nc.vector.memset(warm[:], 0.0)
            nc.scalar.activation(warm[:], warm[:], RELU)

            x_tiles = {}    # (image, chunk group) -> [128, K_TILES, 2*CHUNK]
            ss_tiles = {}   # image -> SS tile
            out_stage = [None] * 4

            def group_tile(n, g):
                w = min(2 * CHUNK, HW - 2 * g * CHUNK)
                t = xpool.tile([128, K_TILES, w], xdt, tag="xc",
                               name=f"xc_{n}_{g}")
                x_tiles[(n, g)] = t
                return t

            def load_piece(eng, n, j, khalf):
                # one [128, 2, 448] quadrant of a chunk on the given queue
                g, half = j // 2, j % 2
                t = x_tiles.get((n, g))
                if t is None:
                    t = group_tile(n, g)
                c0 = half * CHUNK
                k0 = khalf * 2
                eng.dma_start(
                    t[:, k0 : k0 + 2, c0 : c0 + CHUNK],
                    x_d[n, :, k0 : k0 + 2,
                        j * CHUNK : (j + 1) * CHUNK],
                )

            def load_chunk(eng, n, j):
                g, half = j // 2, j % 2
                t = x_tiles.get((n, g))
                if t is None:
                    t = group_tile(n, g)
                c0 = half * CHUNK
                eng.dma_start(
                    t[:, :, c0 : c0 + CHUNK],
                    x_d[n, :, :, j * CHUNK : (j + 1) * CHUNK],
                )

            def load_group(n, g, eng=None):
                eng = eng or nc.gpsimd
                w = min(2 * CHUNK, HW - 2 * g * CHUNK)
                t = group_tile(n, g)
                eng.dma_start(t[:], x_d[n, :, :, 2 * g * CHUNK : 2 * g * CHUNK + w])

            # ---- fill ----
            # The sync HWDGE queue is reserved for outputs: 12.85MB of
            # results drain through it, and any x staged ahead of them
            # delays the final output by that transfer time (measured as a
            # +15us tail regression). All x goes on the gpsimd SWDGE queue
            # (it has ~40us of slack), weights on sync (wsq, tiny, first)
            # and the slow-but-adequate scalar HWDGE queue (w1/w3/bias).
            nc.sync.dma_start(wsq_t[:], wsq_d[:])
            nc.scalar.dma_start(bias_t[:], bias_d[:])
            nc.scalar.dma_start(w1_t[:], w1_d[:])
            nc.scalar.dma_start(w3_t[:], w3_d[:])
            for j in range(N_CHUNK):           # image 0 per-chunk
                load_chunk(nc.gpsimd, 0, j)
            load_group(1, 0, eng=nc.gpsimd)
            load_group(1, 1, eng=nc.gpsimd)
            # Remaining groups issue in-loop, paced ~5 groups ahead of
            # consumption: every group has its own SBUF buffer (xpool
            # bufs=16) so loads never wait on PE progress (WAR), but the
            # tile framework recycles DMA semaphores with cumulative
            # thresholds, so issuing far ahead of consumption would create
            # false waits on much-later DMAs.
            rest_groups = [(1, 2), (1, 3), (2, 0), (2, 1), (2, 2), (2, 3),
                           (3, 0), (3, 1), (3, 2), (3, 3)]
            first_use = {g: 7 * g[0] + 2 * g[1] for g in rest_groups}

            # ---- PE warm-up ----
            for d in range(N_WARM):
                ps = psum.tile([128, ROWS_PER_CHUNK, W], F32, tag="sq", bufs=2,
                               name=f"warm{d}")
                nc.tensor.matmul(ps[:], wdum[:], xdum[:], start=True, stop=True)

            def setup_image(n):
                ss = sspool.tile([128, HP, WP], edt, tag="ss")
                # zero the one-pixel border of the padded S frame (memset
                # rejects the f32r dtype tag, so write through a plain view)
                sdt = BF16 if exp_bf16 else F32
                nc.vector.memset(ss[:, 0, :].bitcast(sdt), 0.0)
                nc.vector.memset(ss[:, HP - 1, :].bitcast(sdt), 0.0)
                nc.vector.memset(ss[:, 1 : HP - 1, 0].bitcast(sdt), 0.0)
                nc.vector.memset(ss[:, 1 : HP - 1, WP - 1].bitcast(sdt), 0.0)
                ss_tiles[n] = ss

            def squeeze_chunk(n, j):
                if n not in ss_tiles:
                    setup_image(n)
                ps = psum.tile([128, ROWS_PER_CHUNK, W], F32, tag="sq", bufs=2,
                               name=f"sq_{n}_{j}")
                xt = x_tiles[(n, j // 2)]
                c0 = (j % 2) * CHUNK
                for k in range(K_TILES):
                    nc.tensor.matmul(
                        ps[:],
                        wsq_t[:, k, :],
                        xt[:, k, c0 : c0 + CHUNK],
                        start=(k == 0),
                        stop=(k == K_TILES - 1),
                    )
                # relu+bias eviction into both duplicated halves of SS
                # interior; alternate DVE/ACT by chunk parity so consecutive
                # evictions overlap instead of queuing on one engine.
                # (even chunks -> DVE: the scalar engine is still issuing
                # fill DMAs when the first eviction retires)
                y0 = j * ROWS_PER_CHUNK
                dst = ss_tiles[n][:, 1 + y0 : 1 + y0 + ROWS_PER_CHUNK, 1 : 1 + W]
                if j % 2 == 0:
                    nc.vector.tensor_scalar(
                        dst, ps[:], bsq_t, 0.0,
                        op0=mybir.AluOpType.add, op1=mybir.AluOpType.max,
                    )
                else:
                    nc.scalar.activation(dst, ps[:], RELU, bias=bsq_t)

            e_state = {}

            def expand_chunk_mm(n, j):
                ss = ss_tiles[n]
                y0 = j * ROWS_PER_CHUNK
                p1 = [psum.tile([128, CHUNK], F32, tag=f"e1h{h}", bufs=1,
                                name=f"p1h{h}_{n}_{j}")
                      for h in range(2)]
                p3 = [psum.tile([128, CHUNK], F32, tag=f"e3h{h}", bufs=2,
                                name=f"p3h{h}_{n}_{j}")
                      for h in range(2)]
                e_state[(n, j)] = (p1, p3)
                # expand1x1: one K=64 matmul per half, concurrent pair
                for h in range(2):
                    nc.tensor.matmul(
                        p1[h][:],
                        w1_t[64 * h : 64 * h + 64, :],
                        ss[64 * h : 64 * h + 64,
                           1 + y0 : 1 + y0 + ROWS_PER_CHUNK, 1 : 1 + W],
                        start=True,
                        stop=True,
                    )
                # expand3x3: 9 shifted taps accumulate; h0/h1 issued as pairs
                for t in range(9):
                    dy, dx = t // 3, t % 3
                    for h in range(2):
                        nc.tensor.matmul(
                            p3[h][:],
                            w3_t[64 * h : 64 * h + 64, t, :],
                            ss[64 * h : 64 * h + 64,
                               y0 + dy : y0 + dy + ROWS_PER_CHUNK,
                               dx : dx + W],
                            start=(t == 0),
                            stop=(t == 8),
                        )

            def expand_chunk_evict(n, j, last=False):
                p1, p3 = e_state.pop((n, j))
                # evictions: e1 on vector engine, e3 on scalar engine (the
                # last chunk's second e3 half goes to DVE to shorten the
                # drain). Outputs stage in 2-chunk tiles; one DMA per role
                # per pair of chunks (issued after the odd chunk's eviction).
                g, half = j // 2, j % 2
                gw = 1 if j == N_CHUNK - 1 else 2   # odd last chunk: solo group
                if half == 0:
                    for role in range(4):
                        out_stage[role] = opool.tile(
                            [128, gw, CHUNK], odt, tag=f"o{role}",
                            name=f"o{role}_{n}_{g}")
                for h in range(2):
                    nc.vector.tensor_scalar(
                        out_stage[h][:, half, :], p1[h][:],
                        b1_t[:, h : h + 1], 0.0,
                        op0=mybir.AluOpType.add, op1=mybir.AluOpType.max,
                    )
                nc.scalar.activation(out_stage[2][:, half, :],
                                     p3[0][:], RELU, bias=b3_t[:, 0:1])
                if last:
                    nc.vector.tensor_scalar(
                        out_stage[3][:, half, :], p3[1][:],
                        b3_t[:, 1:2], 0.0,
                        op0=mybir.AluOpType.add, op1=mybir.AluOpType.max,
                    )
                else:
                    nc.scalar.activation(out_stage[3][:, half, :],
                                         p3[1][:], RELU, bias=b3_t[:, 1:2])
                if half + 1 == gw:
                    # role 3 drains through the scalar HWDGE queue: it
                    # halves nothing else, and taking 3.2MB off the sync
                    # queue pulls the final output completion ~8us earlier
                    engines = [nc.sync, nc.sync, nc.sync, nc.scalar]
                    if last:
                        engines = [nc.sync, nc.sync, nc.scalar, nc.gpsimd]
                    for role in range(4):
                        ch0 = 128 * role
                        engines[role].dma_start(
                            out_d[n, ch0 : ch0 + 128,
                                  2 * g * CHUNK : (2 * g + gw) * CHUNK],
                            out_stage[role][:],
                        )

            # Pipeline: squeeze runs 4-5 chunks ahead of expand, emitted in
            # 2-chunk batches so the PE only pays the 64<->128-row stationary
            # reconfiguration twice per 2 chunks. expand(i)'s dy>=1 taps read
            # S rows written by squeeze(i+1)'s eviction; the 4-chunk lead
            # keeps the PE busy while those evictions retire.
            chunks = [(n, j) for n in range(N_IMG) for j in range(N_CHUNK)]
            NC_ALL = len(chunks)
            # Fill ramp at squeeze-lead 2: interleave the first two expands
            # between squeezes so the PE always has expand work (which needs
            # no fresh x) queued behind a possibly-late x chunk. Batches of
            # [sq sq | e e] only from chunk 2 on, once the fill has caught up.
            squeeze_chunk(*chunks[0])
            squeeze_chunk(*chunks[1])
            squeeze_chunk(*chunks[2])
            expand_chunk_mm(*chunks[0])
            expand_chunk_evict(*chunks[0])
            squeeze_chunk(*chunks[3])
            expand_chunk_mm(*chunks[1])
            expand_chunk_evict(*chunks[1])
            next_rest = 0
            for i in range(2, NC_ALL, 2):
                while (next_rest < len(rest_groups)
                       and first_use[rest_groups[next_rest]] < i + 12):
                    load_group(*rest_groups[next_rest])
                    next_rest += 1
                if i + 2 < NC_ALL:
                    squeeze_chunk(*chunks[i + 2])
                if i + 3 < NC_ALL:
                    squeeze_chunk(*chunks[i + 3])
                last = i + 2 >= NC_ALL
                expand_chunk_mm(*chunks[i])
                expand_chunk_evict(*chunks[i], last=False)
                expand_chunk_mm(*chunks[i + 1])
                expand_chunk_evict(*chunks[i + 1], last=last)

    nc.compile()
    return nc


_NC_CACHE = {}


def _get_nc(in_bf16=IN_BF16, exp_bf16=EXP_BF16, out_bf16=OUT_BF16):
    key = (in_bf16, exp_bf16, out_bf16)
    if key not in _NC_CACHE:
        _NC_CACHE[key] = _build(in_bf16, exp_bf16, out_bf16)
    return _NC_CACHE[key]


def _pack_inputs(x, squeeze_w, squeeze_b, expand1x1_w, expand1x1_b,
                 expand3x3_w, expand3x3_b, in_bf16=IN_BF16, exp_bf16=EXP_BF16):
    """Host-side packing of weights into the SBUF-ready layouts."""
    f = np.float32
    xdt = ml_dtypes.bfloat16 if in_bf16 else f
    edt = ml_dtypes.bfloat16 if exp_bf16 else f
    # wsq[p, k, m] = squeeze_w[m % 64, 128k + p]  (M duplicated 64 -> 128)
    wsq = np.ascontiguousarray(
        np.tile(squeeze_w, (2, 1))                 # [128, 512]
        .T.reshape(K_TILES, 128, 128)              # [k, p, m]
        .transpose(1, 0, 2)
    ).astype(xdt)
    # w1[64h + s, m] = expand1x1_w[128h + m, s]
    w1 = np.concatenate(
        [expand1x1_w[:128].T, expand1x1_w[128:].T], axis=0
    ).astype(edt)                                   # [128, 128]
    # w3[64h + s, 3dy + dx, m] = expand3x3_w[128h + m, s, dy, dx]
    w3e = expand3x3_w.reshape(2, 128, C_SQ, 9)      # [h, m, s, t]
    w3 = np.ascontiguousarray(w3e.transpose(0, 2, 3, 1)).reshape(128, 9, 128)
    w3 = w3.astype(edt)
    # bias[p] = [bsq | b1 pair | b3 pair]  (one DMA for all biases)
    bias = np.empty((128, 5), dtype=f)
    bias[:, 0] = np.tile(squeeze_b, 2)
    bias[:, 1:3] = expand1x1_b.reshape(2, 128).T
    bias[:, 3:5] = expand3x3_b.reshape(2, 128).T
    # [cores, n, 128k+p, s] -> [cores, n, p, k, s] so a chunk-group load is
    # one DMA with partition-major layout
    xs = np.ascontiguousarray(
        x.reshape(N_CORES, N_IMG, K_TILES, 128, HW).transpose(0, 1, 3, 2, 4)
    ).astype(xdt)
    return xs, {"wsq": wsq, "w1": w1, "w3": w3, "bias": bias}


def _run(inputs, trace=False, in_bf16=IN_BF16, exp_bf16=EXP_BF16,
         out_bf16=OUT_BF16):
    from concourse import bass_utils

    nc = _get_nc(in_bf16, exp_bf16, out_bf16)
    xs, weights = _pack_inputs(**inputs, in_bf16=in_bf16, exp_bf16=exp_bf16)
    in_maps = [{"x": xs[c], **weights} for c in range(N_CORES)]
    res = bass_utils.run_bass_kernel_spmd(
        nc, in_maps, core_ids=list(range(N_CORES)), trace=trace
    )
    out = np.concatenate([res.results[c]["out"] for c in range(N_CORES)], axis=0)
    return out.reshape(N_TOTAL, 2 * C_E, H, W).astype(np.float32), res


def kernel(**inputs) -> np.ndarray:
    inputs = {k: np.asarray(v, dtype=np.float32) for k, v in inputs.items()}
    out, _ = _run(inputs, trace=False)
    return out
